# revision 10
# baseline (speedup 1.0000x reference)
"""DeltaAttention Trainium2 kernel — 8-core SPMD via bass/Tile.

Math (per reference): 4 DeltaResidualBlocks (d_v=1) wrapped around MHA.
Because each delta block consumes its v_in only through the scalar
projection v_in @ dWv[i], the Wq/Wk/Wv/Wo matmuls collapse into single
extra columns of the dWk matmuls (precomputed on host), and attn@v
collapses to 2 output columns per head:
    n_h[q] = E_h[q,:] @ u_h,  r_h[q] = E_h[q,:] @ 1,  u_h = v_h @ w_h
    v3[q]  = sum_h n_h/r_h + const,   w = Wo @ dWv[3]
Sharding: 512 query tokens per core; k^T and u are AllGathered within
each 4-core batch group.
"""

import os
from contextlib import ExitStack

import numpy as np
import ml_dtypes

import concourse.bass as bass
import concourse.mybir as mybir
import concourse.tile as tile
from concourse.bass_utils import run_bass_kernel_spmd
from concourse.masks import make_identity

dt = mybir.dt
AF = mybir.ActivationFunctionType
ALU = mybir.AluOpType
ts = bass.ts

N_CORES = 8
B, S, D, H = 2, 2048, 1024, 16
HD = D // H
TOK = (B * S) // N_CORES          # 512 query tokens per core
M4 = TOK // 128                   # 4 token chunks
K8 = D // 128                     # 8 feature chunks
NKC = S // 128                    # 16 key chunks per batch
GROUP = TOK // 128                # token chunks contributed per core to AG
EPS = 1e-8
LN_EPS = 1e-5

# augmented weight widths: dWk (1024) + dbw col + vw col (+ Wu/A,B for i=2)
W_AUG = [1026, 1026, 1058, 1025]

LAST_RESULTS = None
_CACHE = {}


def _split_multi_waits(nc, max_waits=1):
    """walrus (CoreV3) only encodes one sync wait per instruction; Tile's
    final drain can carry several. Hoist extras onto preceding NoOps."""
    n_fixed = 0
    for f in nc.m.functions:
        for blk in f.blocks:
            new_insts = []
            for inst in blk.instructions:
                si = inst.sync_info
                waits = list(si.on_wait) if (si and si.on_wait) else []
                if len(waits) > max_waits:
                    head, tail = waits[:-max_waits], waits[-max_waits:]
                    for j, w in enumerate(head):
                        nop = mybir.InstNoOp(
                            name=f"{inst.name}_waitsplit_{j}",
                            engine=inst.engine,
                            ins=[],
                            outs=[],
                            sync_info=mybir.SyncInfo(on_wait=[w], on_update=[]),
                        )
                        nc.register_instruction(nop)
                        new_insts.append(nop)
                        n_fixed += 1
                    si.on_wait = tail
                new_insts.append(inst)
            blk.instructions[:] = new_insts
    return n_fixed


def _build_program():
    nc = bass.Bass(num_devices=N_CORES)

    x_t = nc.dram_tensor("x", [TOK, D], dt.float32, kind="ExternalInput")
    aug_t = [
        nc.dram_tensor(f"aug{i}", [D, W_AUG[i]], dt.bfloat16, kind="ExternalInput")
        for i in range(4)
    ]
    cvec_t = nc.dram_tensor("cvec", [128, 16], dt.float32, kind="ExternalInput")
    lng_t = nc.dram_tensor("lng", [128, D], dt.float32, kind="ExternalInput")
    lnb_t = nc.dram_tensor("lnb", [128, D], dt.float32, kind="ExternalInput")
    y_t = nc.dram_tensor("y", [TOK, D], dt.float32, kind="ExternalOutput")

    RG = [[0, 1, 2, 3], [4, 5, 6, 7]]

    with tile.TileContext(nc) as tc, ExitStack() as stack:
        const = stack.enter_context(tc.tile_pool(name="const", bufs=1))
        dram = stack.enter_context(tc.tile_pool(name="dram", bufs=1, space="DRAM"))
        big = stack.enter_context(tc.tile_pool(name="big", bufs=1))

        agk_in = dram.tile([D, TOK], dt.bfloat16, tag="agk_in")
        agk_out = dram.tile([4 * D, TOK], dt.bfloat16, tag="agk_out")
        agu_in = dram.tile([TOK, H], dt.bfloat16, tag="agu_in")
        agu_out = dram.tile([4 * TOK, H], dt.bfloat16, tag="agu_out")

        ident_bf = const.tile([128, 128], dt.bfloat16, tag="ident_bf")
        make_identity(nc, ident_bf[:])
        ident_f32 = const.tile([128, 128], dt.float32, tag="ident_f32")
        make_identity(nc, ident_f32[:])
        cvec = const.tile([128, 16], dt.float32, tag="cvec")
        nc.sync.dma_start(cvec[:], cvec_t[:])
        lng = const.tile([128, D], dt.float32, tag="lng")
        lnb = const.tile([128, D], dt.float32, tag="lnb")
        nc.sync.dma_start(lng[:], lng_t[:])
        nc.sync.dma_start(lnb[:], lnb_t[:])

        # persistent data tiles
        x32 = [big.tile([128, D], dt.float32, tag=f"x32_{m}", name=f"x32_{m}") for m in range(M4)]
        xbf = [big.tile([128, D], dt.bfloat16, tag=f"xbf_{m}", name=f"xbf_{m}") for m in range(M4)]
        xT = [big.tile([128, TOK], dt.bfloat16, tag=f"xT_{k}", name=f"xT_{k}") for k in range(K8)]
        qT = [big.tile([128, TOK], dt.bfloat16, tag=f"qT_{k}", name=f"qT_{k}") for k in range(K8)]
        k3raw = [big.tile([128, D], dt.bfloat16, tag=f"k3_{m}", name=f"k3_{m}") for m in range(M4)]
        a3s = [big.tile([128, 1], dt.float32, tag=f"a3_{m}", name=f"a3_{m}") for m in range(M4)]
        b3s = [big.tile([128, 1], dt.float32, tag=f"b3_{m}", name=f"b3_{m}") for m in range(M4)]
        u_bf = [big.tile([128, H], dt.bfloat16, tag=f"u_{m}", name=f"u_{m}") for m in range(M4)]
        nr_sb = big.tile([2, H * TOK], dt.float32, tag="nr_sb")

        for m in range(M4):
            nc.sync.dma_start(x32[m][:], x_t[ts(m, 128), :])
            nc.scalar.copy(xbf[m][:], x32[m][:])

        with (
            tc.tile_pool(name="wpool", bufs=16) as wpool,
            tc.tile_pool(name="qkpool", bufs=4) as qkpool,
            tc.tile_pool(name="scpool", bufs=24) as scpool,
            tc.tile_pool(name="scr", bufs=2) as scrpool,
            tc.tile_pool(name="ktloc", bufs=8) as ktlpool,
            tc.tile_pool(name="pp_proj", bufs=2, space="PSUM") as pp_proj,
            tc.tile_pool(name="pp_t", bufs=2, space="PSUM") as pp_t,
        ):
            # x^T via PE transpose (bf16)
            for k in range(K8):
                pst = pp_t.tile([128, TOK], dt.bfloat16, tag="pst")
                for m in range(M4):
                    nc.tensor.transpose(
                        pst[:, ts(m, 128)], xbf[m][:, ts(k, 128)], ident_bf[:]
                    )
                nc.vector.tensor_copy(xT[k][:], pst[:])

            qk_out = {}  # aug idx -> list of bf16 (128, D) tiles

            def delta_block(i):
                """matmuls + delta elementwise for aug i on all 4 token chunks."""
                W = W_AUG[i]
                augt = [
                    wpool.tile([128, W_AUG[2]], dt.bfloat16, tag="aug", name=f"aug_{i}_{_k}")[:, 0:W]
                    for _k in range(K8)
                ]
                for k in range(K8):
                    nc.sync.dma_start(augt[k][:], aug_t[i][ts(k, 128), :])
                strips = [(0, 512), (512, 1024), (1024, W)]
                outs = []
                for m in range(M4):
                    ps = pp_proj.tile([128, 1536], dt.float32, tag="ps_proj")
                    for k in range(K8):
                        for (s0, s1) in strips:
                            nc.tensor.matmul(
                                ps[:, s0:s1],
                                xT[k][:, ts(m, 128)],
                                augt[k][:, s0:s1],
                                start=(k == 0),
                                stop=(k == K8 - 1),
                            )
                    # ---- delta elementwise for this chunk
                    scr = scrpool.tile([128, D], dt.bfloat16, tag="scr")
                    ss = scpool.tile([128, 1], dt.float32, tag="sc")
                    nc.scalar.activation(scr[:], ps[:, 0:D], AF.Square, accum_out=ss[:])
                    kx1 = scpool.tile([128, 1], dt.float32, tag="sc")
                    kx2 = scpool.tile([128, 1], dt.float32, tag="sc")
                    kx = scpool.tile([128, 1], dt.float32, tag="sc")
                    scr2 = scrpool.tile([128, D], dt.bfloat16, tag="scr", name=f"scr2_{i}_{m}")
                    nc.vector.scalar_tensor_tensor(
                        scr2[:, 0:512], ps[:, 0:512], 1.0, x32[m][:, 0:512],
                        ALU.mult, ALU.mult, accum_out=kx1[:],
                    )
                    nc.vector.scalar_tensor_tensor(
                        scr2[:, 512:1024], ps[:, 512:1024], 1.0, x32[m][:, 512:1024],
                        ALU.mult, ALU.mult, accum_out=kx2[:],
                    )
                    nc.vector.tensor_tensor(kx[:], kx1[:], kx2[:], ALU.add)
                    # rnorm = 1 / (sqrt(ss) + EPS)   [sqrt via exp(0.5 ln)]
                    lnv = scpool.tile([128, 1], dt.float32, tag="sc")
                    nc.scalar.activation(lnv[:], ss[:], AF.Ln)
                    nrm = scpool.tile([128, 1], dt.float32, tag="sc")
                    nc.scalar.activation(nrm[:], lnv[:], AF.Exp, scale=0.5)
                    nrme = scpool.tile([128, 1], dt.float32, tag="sc")
                    nc.vector.tensor_scalar_add(nrme[:], nrm[:], EPS)
                    rnorm = scpool.tile([128, 1], dt.float32, tag="sc")
                    nc.vector.reciprocal(rnorm[:], nrme[:])
                    # beta/2 = sigmoid(z + dbb) = 1/(1+exp(-z - dbb))
                    ez = scpool.tile([128, 1], dt.float32, tag="sc")
                    nc.scalar.activation(
                        ez[:], ps[:, D:D + 1], AF.Exp, scale=-1.0,
                        bias=cvec[:, i:i + 1],
                    )
                    ez1 = scpool.tile([128, 1], dt.float32, tag="sc")
                    nc.vector.tensor_scalar_add(ez1[:], ez[:], 1.0)
                    rsig = scpool.tile([128, 1], dt.float32, tag="sc")
                    nc.vector.reciprocal(rsig[:], ez1[:])
                    # s = 2*sigmoid * rnorm * (v - rnorm*kx)
                    rk = scpool.tile([128, 1], dt.float32, tag="sc")
                    nc.vector.tensor_scalar_mul(rk[:], kx[:], rnorm[:])
                    dv = scpool.tile([128, 1], dt.float32, tag="sc")
                    if i < 3:
                        v = scpool.tile([128, 1], dt.float32, tag="sc")
                        nc.vector.tensor_scalar_add(
                            v[:], ps[:, D + 1:D + 2], cvec[:, 4 + i:5 + i]
                        )
                        nc.vector.tensor_tensor(dv[:], v[:], rk[:], ALU.subtract)
                    rr = scpool.tile([128, 1], dt.float32, tag="sc")
                    nc.vector.tensor_scalar(rr[:], rsig[:], rnorm[:], 2.0, ALU.mult, ALU.mult)
                    if i < 3:
                        s = scpool.tile([128, 1], dt.float32, tag="sc")
                        nc.vector.tensor_tensor(s[:], dv[:], rr[:], ALU.mult)
                    if i in (0, 1):
                        o = qkpool.tile([128, D], dt.bfloat16, tag="qk")
                        nc.vector.scalar_tensor_tensor(
                            o[:], ps[:, 0:D], s[:], x32[m][:], ALU.mult, ALU.add
                        )
                        outs.append(o)
                    elif i == 2:
                        # u = A + s*B   (A = x@Wu cols, B = x@dWk2@Wu cols)
                        # (DVE allows only one PSUM input: stage A to SBUF)
                        ua = scpool.tile([128, H], dt.float32, tag="ua", name=f"ua_{m}")
                        nc.vector.tensor_copy(ua[:], ps[:, 1026:1042])
                        nc.vector.scalar_tensor_tensor(
                            u_bf[m][:], ps[:, 1042:1058], s[:], ua[:],
                            ALU.mult, ALU.add,
                        )
                    else:
                        # stash k3_raw and the scalars for s3 = a3*v3 - b3
                        nc.vector.tensor_scalar_mul(k3raw[m][:], ps[:, 0:D], 1.0)
                        nc.vector.tensor_copy(a3s[m][:], rr[:])
                        nc.vector.tensor_tensor(b3s[m][:], rr[:], rk[:], ALU.mult)
                qk_out[i] = outs

            def transpose_to(src_tiles, dst_tiles):
                for k in range(K8):
                    pst = pp_t.tile([128, TOK], dt.bfloat16, tag="pst")
                    for m in range(M4):
                        nc.tensor.transpose(
                            pst[:, ts(m, 128)], src_tiles[m][:, ts(k, 128)], ident_bf[:]
                        )
                    nc.vector.tensor_copy(dst_tiles[k][:], pst[:])

            # ---- k path first so the AllGather starts early
            delta_block(1)
            ktloc = [ktlpool.tile([128, TOK], dt.bfloat16, tag="ktloc", name=f"ktloc_{_k}") for _k in range(K8)]
            transpose_to(qk_out[1], ktloc)
            for k in range(K8):
                nc.sync.dma_start(agk_in[ts(k, 128), :], ktloc[k][:])
            delta_block(2)
            for m in range(M4):
                nc.sync.dma_start(agu_in[ts(m, 128), :], u_bf[m][:])
            nc.gpsimd.collective_compute(
                "AllGather", ALU.bypass, ins=[agk_in[:]], outs=[agk_out[:]],
                replica_groups=RG,
            )
            nc.gpsimd.collective_compute(
                "AllGather", ALU.bypass, ins=[agu_in[:]], outs=[agu_out[:]],
                replica_groups=RG,
            )
            delta_block(0)
            transpose_to(qk_out[0], qT)
            delta_block(3)

        # ---------------- attention ----------------
        with (
            tc.tile_pool(name="attn_sb", bufs=1) as attn_sb,
            tc.tile_pool(name="epool", bufs=3) as epool,
            tc.tile_pool(name="pp_sc", bufs=3, space="PSUM") as pp_sc,
            tc.tile_pool(name="pp_nr", bufs=2, space="PSUM") as pp_nr,
        ):
            kT = [attn_sb.tile([128, S], dt.bfloat16, tag=f"kT_{k}", name=f"kTsb_{k}") for k in range(K8)]
            for k in range(K8):
                for c in range(4):
                    nc.sync.dma_start(
                        kT[k][:, ts(c, TOK)], agk_out[c * D + 128 * k: c * D + 128 * (k + 1), :]
                    )
            uext = [attn_sb.tile([128, 2 * H], dt.bfloat16, tag=f"ue_{kc}", name=f"ue_{kc}") for kc in range(NKC)]
            for kc in range(NKC):
                nc.vector.memset(uext[kc][:], 1.0)
                nc.sync.dma_start(
                    uext[kc][:].rearrange("p (h two) -> p h two", two=2)[:, :, 0],
                    agu_out[ts(kc, 128), :],
                )

            for hp in range(K8):  # 8 head pairs; pair hp = heads 2hp, 2hp+1
                nr_ps = pp_nr.tile([128, TOK], dt.float32, tag="nr")
                for kc in range(NKC):
                    ps2 = pp_sc.tile([128, 2, TOK], dt.float32, tag="sc2")
                    nc.tensor.matmul(
                        ps2[:, 0, :], kT[hp][0:64, ts(kc, 128)], qT[hp][0:64, :],
                        start=True, stop=True, tile_position=(0, 0),
                    )
                    nc.tensor.matmul(
                        ps2[:, 1, :], kT[hp][64:128, ts(kc, 128)], qT[hp][64:128, :],
                        start=True, stop=True, tile_position=(64, 0),
                    )
                    E = epool.tile([128, 2, TOK], dt.bfloat16, tag="E")
                    nc.scalar.activation(E[:], ps2[:], AF.Exp, scale=float(HD) ** -0.5)
                    nc.tensor.matmul(
                        nr_ps[0:2, :], uext[kc][:, 2 * (2 * hp): 2 * (2 * hp) + 2],
                        E[:, 0, :], start=(kc == 0), stop=(kc == NKC - 1),
                        tile_position=(0, 0),
                    )
                    nc.tensor.matmul(
                        nr_ps[32:34, :], uext[kc][:, 2 * (2 * hp + 1): 2 * (2 * hp + 1) + 2],
                        E[:, 1, :], start=(kc == 0), stop=(kc == NKC - 1),
                        tile_position=(0, 32),
                    )
                h0, h1 = 2 * hp, 2 * hp + 1
                nc.vector.tensor_copy(nr_sb[0:2, h0 * TOK:(h0 + 1) * TOK], nr_ps[0:2, :])
                nc.vector.tensor_copy(nr_sb[0:2, h1 * TOK:(h1 + 1) * TOK], nr_ps[32:34, :])

            # ---- v3, final delta, layernorm
            with tc.tile_pool(name="fin", bufs=2) as fin:
                for m in range(M4):
                    psT = pp_sc.tile([128, 2, TOK], dt.float32, tag="sc2")
                    for h in range(H):
                        nc.tensor.transpose(
                            psT[:, 0, 2 * h:2 * h + 2],
                            nr_sb[0:2, h * TOK + 128 * m: h * TOK + 128 * (m + 1)],
                            ident_f32[0:2, 0:2],
                        )
                    # free-dim layout after transpose: [n(h0), r(h0), n(h1), r(h1), ...]
                    nrT = fin.tile([128, 32], dt.float32, tag="nrT")
                    nc.vector.tensor_copy(nrT[:], psT[:, 0, 0:32])
                    rec = fin.tile([128, H], dt.float32, tag="rec")
                    nc.vector.reciprocal(rec[:], nrT[:, 1:32:2])
                    prod = fin.tile([128, H], dt.float32, tag="prod")
                    nc.vector.tensor_tensor(prod[:], nrT[:, 0:32:2], rec[:], ALU.mult)
                    v3p = fin.tile([128, 1], dt.float32, tag="v3p")
                    nc.vector.tensor_reduce(v3p[:], prod[:], axis=mybir.AxisListType.X, op=ALU.add)
                    v3 = fin.tile([128, 1], dt.float32, tag="v3")
                    nc.vector.tensor_scalar_add(v3[:], v3p[:], cvec[:, 7:8])
                    s3 = fin.tile([128, 1], dt.float32, tag="s3")
                    nc.vector.tensor_scalar_mul(s3[:], v3[:], a3s[m][:])
                    nc.vector.tensor_tensor(s3[:], s3[:], b3s[m][:], ALU.subtract)
                    y32 = fin.tile([128, D], dt.float32, tag="y32")
                    nc.vector.scalar_tensor_tensor(
                        y32[:], k3raw[m][:], s3[:], x32[m][:], ALU.mult, ALU.add
                    )
                    # layernorm
                    mu = fin.tile([128, 1], dt.float32, tag="mu")
                    nc.vector.tensor_reduce(mu[:], y32[:], axis=mybir.AxisListType.X, op=ALU.add)
                    nc.vector.tensor_scalar_mul(mu[:], mu[:], 1.0 / D)
                    ssy = fin.tile([128, 1], dt.float32, tag="ssy")
                    scr3 = fin.tile([128, D], dt.bfloat16, tag="scr3")
                    nc.scalar.activation(scr3[:], y32[:], AF.Square, accum_out=ssy[:])
                    mu2 = fin.tile([128, 1], dt.float32, tag="mu2")
                    nc.vector.tensor_tensor(mu2[:], mu[:], mu[:], ALU.mult)
                    var = fin.tile([128, 1], dt.float32, tag="var")
                    nc.vector.tensor_scalar_mul(var[:], ssy[:], 1.0 / D)
                    nc.vector.tensor_tensor(var[:], var[:], mu2[:], ALU.subtract)
                    nc.vector.tensor_scalar_add(var[:], var[:], LN_EPS)
                    lnv2 = fin.tile([128, 1], dt.float32, tag="lnv2")
                    nc.scalar.activation(lnv2[:], var[:], AF.Ln)
                    rstd = fin.tile([128, 1], dt.float32, tag="rstd")
                    nc.scalar.activation(rstd[:], lnv2[:], AF.Exp, scale=-0.5)
                    yn = fin.tile([128, D], dt.float32, tag="yn")
                    nc.vector.tensor_scalar(yn[:], y32[:], mu[:], rstd[:], ALU.subtract, ALU.mult)
                    yg = fin.tile([128, D], dt.float32, tag="yg")
                    nc.vector.tensor_tensor(yg[:], yn[:], lng[:], ALU.mult)
                    nc.vector.tensor_tensor(yg[:], yg[:], lnb[:], ALU.add)
                    nc.sync.dma_start(y_t[ts(m, 128), :], yg[:])

    _split_multi_waits(nc)
    nc.finalize()
    return nc


def _host_prep(inputs):
    """Precompute augmented weights and constants; returns per-core in_maps."""
    f32 = np.float32
    x = np.asarray(inputs["x"], f32)
    Wq, bq = np.asarray(inputs["Wq"], f32), np.asarray(inputs["bq"], f32)
    Wk, bk = np.asarray(inputs["Wk"], f32), np.asarray(inputs["bk"], f32)
    Wv, bv = np.asarray(inputs["Wv"], f32), np.asarray(inputs["bv"], f32)
    Wo, bo = np.asarray(inputs["Wo"], f32), np.asarray(inputs["bo"], f32)
    dWk, dbw = np.asarray(inputs["dWk"], f32), np.asarray(inputs["dbw"], f32)
    dbb, dWv = np.asarray(inputs["dbb"], f32), np.asarray(inputs["dWv"], f32)
    dbv = np.asarray(inputs["dbv"], f32)
    ln_g, ln_b = np.asarray(inputs["ln_g"], f32), np.asarray(inputs["ln_b"], f32)

    w = Wo @ dWv[3]                                   # (D,)
    Wu = np.zeros((D, H), f32)
    for h in range(H):
        Wu[h * HD:(h + 1) * HD, h] = w[h * HD:(h + 1) * HD]
    Bu = dWk[2] @ Wu                                  # (D, H)

    vw = [Wq @ dWv[0], Wk @ dWv[1], Wv @ dWv[2]]
    vc = [float(bq @ dWv[0] + dbv[0]), float(bk @ dWv[1] + dbv[1]),
          float(bv @ dWv[2] + dbv[2])]
    c3 = float(bo @ dWv[3] + dbv[3])

    bf = ml_dtypes.bfloat16
    augs = []
    for i in range(4):
        cols = [dWk[i], dbw[i][:, None]]
        if i < 3:
            cols.append(vw[i][:, None])
        if i == 2:
            cols.append(Wu)
            cols.append(Bu)
        augs.append(np.ascontiguousarray(np.concatenate(cols, axis=1)).astype(bf))
        assert augs[i].shape[1] == W_AUG[i], (i, augs[i].shape)

    cvec = np.zeros((128, 16), f32)
    for i in range(4):
        cvec[:, i] = -dbb[i]
    for i in range(3):
        cvec[:, 4 + i] = vc[i]
    cvec[:, 7] = c3

    lng = np.broadcast_to(ln_g[None, :], (128, D)).copy()
    lnb = np.broadcast_to(ln_b[None, :], (128, D)).copy()

    xf = x.reshape(B * S, D)
    in_maps = []
    for c in range(N_CORES):
        m = {
            "x": np.ascontiguousarray(xf[c * TOK:(c + 1) * TOK]),
            "cvec": cvec, "lng": lng, "lnb": lnb,
        }
        for i in range(4):
            m[f"aug{i}"] = augs[i]
        in_maps.append(m)
    return in_maps


def kernel(**inputs):
    global LAST_RESULTS
    if "nc" not in _CACHE:
        _CACHE["nc"] = _build_program()
    nc = _CACHE["nc"]
    in_maps = _host_prep(inputs)
    res = run_bass_kernel_spmd(nc, in_maps, core_ids=list(range(N_CORES)))
    LAST_RESULTS = res
    out = np.concatenate(
        [res.results[c]["y"] for c in range(N_CORES)], axis=0
    ).reshape(B, S, D)
    return out.astype(np.float32)
